# revision 16
# baseline (speedup 1.0000x reference)
"""Deformable KPConv layer on 8 Trainium2 NeuronCores (Bass/Tile).

Strategy (data-parallel over the 16384 query points, 2048/core):
  - features pre-cast to fp16 host-side and pregathered per-edge into an
    "edge-slot" layout [(4 queries x 32 neighbors) partitions, group, 128 feat].
  - relative neighbor coords (s - q) prepacked fp16 d-major host-side in
    query-partition layout; squared distances to the (possibly deformed)
    kernel points are computed difference-first (|rel - C|^2 with
    C = kp + offset) on DVE/GpSimd in fp16 -- no cancellation, sq >= 0 by
    construction. Influence w' = min(d,2) - 2 (sign+1/2 folded into the
    conv weights host-side).
  - neighbor contraction on TensorE as block-diagonal matmuls: w' is
    scattered into a zero-initialized block-diagonal SBUF tile via a DRAM
    bounce (partition remap), then psum[f,(q,k)] = nf^T @ wblk.
  - PSUM drains are flat contiguous copies on the Scalar engine into
    wf[f, q, k]; the (k,f)->42 offset projection and (k,f)->256 output
    projection are PSUM-accumulated matmuls with strided wf[:, :, k] views
    as stationary operands, producing query-partition outputs directly.
  - emission is software-pipelined two deep: stage 1 of tile t-1 is
    interleaved after stage 0 of tile t, so in-order engine queues always
    have ready work behind a cross-stage dependency wait.
"""

import os
import sys

sys.path.insert(0, "/opt/trn_rl_repo")

import numpy as np

import concourse.bass as bass
import concourse.tile as tile
from concourse import bacc, mybir

N_Q = 16384
N_S = 16384
NN = 32
F_IN = 128
F_OUT = 256
K = 15
DIM = 3
OFF_DIM = DIM * (K - 1)  # 42
N_CORES = 8
P = 128

F16 = mybir.dt.float16
F32 = mybir.dt.float32


def build_nc(qpc: int):
    T = qpc // P  # query tiles per core
    NG = P // 4  # 32 groups of 4 queries per tile

    nc = bacc.Bacc("TRN2", target_bir_lowering=False)

    nfg_d = nc.dram_tensor("nfg", [T, P, NN, F_IN], F16, kind="ExternalInput")
    relg_d = nc.dram_tensor("relg", [T, P, DIM, NN], F16, kind="ExternalInput")
    rele_d = nc.dram_tensor("rele", [T, P, DIM, NN], F16, kind="ExternalInput")
    kprep_d = nc.dram_tensor("kprep", [P, K * DIM], F32, kind="ExternalInput")
    dwsb_d = nc.dram_tensor("dwsb", [P, K * OFF_DIM], F16, kind="ExternalInput")
    wsb_d = nc.dram_tensor("wsb", [P, K * F_OUT], F16, kind="ExternalInput")
    brep_d = nc.dram_tensor("brep", [P, OFF_DIM], F32, kind="ExternalInput")
    out_d = nc.dram_tensor("out", [qpc, F_OUT], F32, kind="ExternalOutput")

    with tile.TileContext(nc) as tc:
        with (
            tc.tile_pool(name="const", bufs=1) as cpool,
            tc.tile_pool(name="nf", bufs=5) as nfpool,
            tc.tile_pool(name="rl", bufs=4) as rlpool,
            tc.tile_pool(name="sq", bufs=3) as sqpool,
            tc.tile_pool(name="wd", bufs=3) as wdpool,
            tc.tile_pool(name="wf", bufs=3) as wfpool,
            tc.tile_pool(name="cc", bufs=3) as ccpool,
            tc.tile_pool(name="outp", bufs=2) as opool,
            tc.tile_pool(name="dram", bufs=6, space="DRAM") as drpool,
            tc.tile_pool(name="ps", bufs=3, space="PSUM") as pspool,
            tc.tile_pool(name="ps2", bufs=2, space="PSUM") as ps2pool,
        ):
            # --- constants, loaded once ---
            kprep = cpool.tile([P, K, DIM], F32, tag="kprep")
            nc.sync.dma_start(out=kprep[:], in_=kprep_d[:].rearrange("p (k d) -> p k d", d=DIM))
            dwsb = cpool.tile([P, K * OFF_DIM], F16, tag="dwsb")
            nc.sync.dma_start(out=dwsb[:], in_=dwsb_d[:])
            wsb = cpool.tile([P, K * F_OUT], F16, tag="wsb")
            nc.sync.dma_start(out=wsb[:], in_=wsb_d[:])
            brep = cpool.tile([P, OFF_DIM], F32, tag="brep")
            nc.sync.dma_start(out=brep[:], in_=brep_d[:])
            eps_c = cpool.tile([P, 1], F32, tag="eps")
            nc.vector.memset(eps_c[:], 1e-6)
            # fp16 kernel points for the stage-0 (rigid) distances
            kh0 = cpool.tile([P, K, DIM], F16, tag="kh0")
            nc.vector.tensor_copy(out=kh0[:], in_=kprep[:])

            # persistent block-diagonal tiles (zeros off-diagonal; only the
            # diagonal blocks are ever overwritten by the scatter DMAs)
            wblks = []
            for i in range(6):
                wb = nc.alloc_sbuf_tensor(f"wblk{i}", [P, NG, 4 * K], F16)
                nc.gpsimd.memset(wb.ap(), 0.0)
                wblks.append(wb)

            # per-tile state carried between pipeline phases
            state = {}

            def emit_load(t):
                nf = nfpool.tile([P, NN, F_IN], F16, tag="nf")
                nc.sync.dma_start(out=nf[:], in_=nfg_d[t])
                rl = rlpool.tile([P, DIM, NN], F16, tag="rl")
                nc.sync.dma_start(out=rl[:], in_=relg_d[t])
                re = rlpool.tile([P, DIM, NN], F16, tag="re")
                nc.sync.dma_start(out=re[:], in_=rele_d[t])
                state[t] = {"nf": nf, "rl": rl, "re": re}

            def emit_sqrtdist(t, rl, ch):
                """fp16 |rel - C|^2 -> d = sqrt(sq) on ScalarE.
                rel rows broadcast over k, C columns broadcast over n/g."""

                def bc_rel(d):
                    return rl[:, d, :].unsqueeze(2).broadcast_to([P, NN, K])

                def bc_c(d):
                    return ch[:, :, d].unsqueeze(1).broadcast_to([P, NN, K])

                dx = sqpool.tile([P, NN, K], F16, tag="dx")
                nc.gpsimd.tensor_tensor(
                    out=dx[:], in0=bc_rel(0), in1=bc_c(0), op=mybir.AluOpType.subtract
                )
                dy = sqpool.tile([P, NN, K], F16, tag="dy")
                nc.gpsimd.tensor_tensor(
                    out=dy[:], in0=bc_rel(1), in1=bc_c(1), op=mybir.AluOpType.subtract
                )
                dz = sqpool.tile([P, NN, K], F16, tag="dz")
                nc.vector.tensor_tensor(
                    out=dz[:], in0=bc_rel(2), in1=bc_c(2), op=mybir.AluOpType.subtract
                )
                mx = sqpool.tile([P, NN, K], F16, tag="mx")
                nc.vector.tensor_tensor(
                    out=mx[:], in0=dx[:], in1=dx[:], op=mybir.AluOpType.mult
                )
                my = sqpool.tile([P, NN, K], F16, tag="my")
                nc.vector.tensor_tensor(
                    out=my[:], in0=dy[:], in1=dy[:], op=mybir.AluOpType.mult
                )
                mz = sqpool.tile([P, NN, K], F16, tag="mz")
                nc.vector.tensor_tensor(
                    out=mz[:], in0=dz[:], in1=dz[:], op=mybir.AluOpType.mult
                )
                a1 = sqpool.tile([P, NN, K], F16, tag="a1")
                nc.gpsimd.tensor_tensor(
                    out=a1[:], in0=mx[:], in1=my[:], op=mybir.AluOpType.add
                )
                sqt = sqpool.tile([P, NN, K], F16, tag="sqt")
                nc.vector.tensor_tensor(
                    out=sqt[:], in0=a1[:], in1=mz[:], op=mybir.AluOpType.add
                )
                dts = wdpool.tile([P, NN, K], F16, tag="dts")
                nc.scalar.activation(
                    out=dts[:], in_=sqt[:],
                    func=mybir.ActivationFunctionType.Sqrt, bias=eps_c[:],
                )
                return dts

            def emit_dists0(t):
                """Stage 0: C = kp is query-independent, so the whole distance
                pipeline runs in edge-slot layout and w' = min(d,2)-2 writes
                the diagonal blocks of wblk in place -- no partition remap."""
                dts = emit_sqrtdist(t, state[t]["re"], kh0)
                wblk = wblks[t % 3].ap()
                engs = [nc.vector, nc.gpsimd, nc.vector, nc.gpsimd]
                for qq in range(4):
                    engs[qq].tensor_scalar(
                        out=wblk[32 * qq : 32 * (qq + 1), :, K * qq : K * (qq + 1)],
                        in0=dts[32 * qq : 32 * (qq + 1), :, :],
                        scalar1=2.0,
                        scalar2=2.0,
                        op0=mybir.AluOpType.min,
                        op1=mybir.AluOpType.subtract,
                    )
                return wblk

            def emit_dists1(t, ch):
                """Stage 1: per-query C -> q-layout compute + DRAM-bounce
                scatter into the block-diagonal tile."""
                dts = emit_sqrtdist(t, state[t]["rl"], ch)
                wdense = wdpool.tile([P, NN * K], F16, tag="wdense")
                nc.vector.tensor_scalar(
                    out=wdense[:],
                    in0=dts[:].rearrange("p n k -> p (n k)"),
                    scalar1=2.0,
                    scalar2=2.0,
                    op0=mybir.AluOpType.min,
                    op1=mybir.AluOpType.subtract,
                )
                wblk = wblks[3 + (t % 3)].ap()
                bounce = drpool.tile([P, NN * K], F16, tag="bounce")
                nc.scalar.dma_start(out=bounce[:], in_=wdense[:])
                wsrc = bounce[:].rearrange("(g qq) (n k) -> qq n g k", qq=4, k=K)
                for qq in range(4):
                    nc.sync.dma_start(
                        out=wblk[32 * qq : 32 * (qq + 1), :, K * qq : K * (qq + 1)],
                        in_=wsrc[qq],
                    )
                return wblk

            def emit_contract(t, stage, wblk):
                """psum[f, (q-in-block, k)] = nf^T . wblk, drained to
                wf[f, q, k] (flat copies on the Scalar engine)."""
                nf = state[t]["nf"]
                wf_sb = wfpool.tile([P, P, K], F16, tag=f"wf{stage}")
                for b in range(4):
                    psb = pspool.tile([P, 8 * 4 * K], F32, tag="psb")
                    for g8 in range(8):
                        g = b * 8 + g8
                        nc.tensor.matmul(
                            out=psb[:, g8 * 60 : (g8 + 1) * 60],
                            lhsT=nf[:, g, :],
                            rhs=wblk[:, g, :],
                            start=True,
                            stop=True,
                        )
                    nc.scalar.activation(
                        out=wf_sb[:, 32 * b : 32 * (b + 1), :],
                        in_=psb[:].rearrange("p (q k) -> p q k", k=K),
                        func=mybir.ActivationFunctionType.Copy,
                    )
                return wf_sb

            def phase_A(t):
                emit_load(t)
                state[t]["wblk0"] = emit_dists0(t)

            def phase_B(t):
                wf0 = emit_contract(t, 0, state[t].pop("wblk0"))
                # offset projection: feat0[q, o] = sum_k wf0_k^T . dw_k
                psA = ps2pool.tile([P, OFF_DIM], F32, tag="psA")
                for k in range(K):
                    nc.tensor.matmul(
                        out=psA[:],
                        lhsT=wf0[:, :, k],
                        rhs=dwsb[:, k * OFF_DIM : (k + 1) * OFF_DIM],
                        start=(k == 0),
                        stop=(k == K - 1),
                    )
                state[t]["psA"] = psA

            def phase_C(t):
                # C1 = kp + offsets (k=0 offset is zero); offsets stay f32
                psA = state[t].pop("psA")
                off_sb = ccpool.tile([P, OFF_DIM], F32, tag="off")
                nc.vector.tensor_tensor(
                    out=off_sb[:], in0=psA[:], in1=brep[:], op=mybir.AluOpType.add
                )
                c1 = ccpool.tile([P, K, DIM], F32, tag="c1")
                nc.vector.tensor_copy(out=c1[:, 0, :], in_=kprep[:, 0, :])
                nc.vector.tensor_tensor(
                    out=c1[:, 1:K, :],
                    in0=kprep[:, 1:K, :],
                    in1=off_sb[:].rearrange("p (k d) -> p k d", d=DIM),
                    op=mybir.AluOpType.add,
                )
                ch1 = ccpool.tile([P, K, DIM], F16, tag="ch1")
                nc.vector.tensor_copy(out=ch1[:], in_=c1[:])
                state[t]["wblk1"] = emit_dists1(t, ch1)

            def phase_D(t):
                wf1 = emit_contract(t, 1, state[t].pop("wblk1"))
                # output projection: out[q, o] = sum_k wf1_k^T . W_k
                psO = ps2pool.tile([P, F_OUT], F32, tag="psO")
                for k in range(K):
                    nc.tensor.matmul(
                        out=psO[:],
                        lhsT=wf1[:, :, k],
                        rhs=wsb[:, k * F_OUT : (k + 1) * F_OUT],
                        start=(k == 0),
                        stop=(k == K - 1),
                    )
                out_sb = opool.tile([P, F_OUT], F32, tag="outsb")
                nc.scalar.activation(
                    out=out_sb[:], in_=psO[:],
                    func=mybir.ActivationFunctionType.Copy,
                )
                nc.scalar.dma_start(
                    out=out_d[t * P : (t + 1) * P, :], in_=out_sb[:]
                )
                del state[t]

            # four-deep software pipeline over tiles; oldest phase emitted
            # first so fresh semaphore waits sit at queue tails
            for step in range(T + 3):
                if 3 <= step < T + 3:
                    phase_D(step - 3)
                if 2 <= step < T + 2:
                    phase_C(step - 2)
                if 1 <= step < T + 1:
                    phase_B(step - 1)
                if step < T:
                    phase_A(step)

    nc.compile()
    return nc


def _prep_shared(support_points, features, K_points, weight, deformable_weight, bias):
    f16 = features.astype(np.float16)
    spT = support_points.T.astype(np.float32)  # [3, N_S]
    kprep = np.broadcast_to(
        K_points.reshape(1, K * DIM), (P, K * DIM)
    ).astype(np.float32).copy()
    dwsb = (
        deformable_weight.transpose(1, 0, 2).reshape(F_IN, K * OFF_DIM) * -0.5
    ).astype(np.float16)
    wsb = (
        weight.transpose(1, 0, 2).reshape(F_IN, K * F_OUT) * -0.5
    ).astype(np.float16)
    brep = np.broadcast_to(bias.reshape(1, OFF_DIM), (P, OFF_DIM)).astype(
        np.float32
    ).copy()
    return f16, spT, kprep, dwsb, wsb, brep


def _prep_core(query_points, neighbors, qpc, f16, spT):
    """Shard-local tensors: pregathered neighbor features (edge-slot layout)
    and relative neighbor coords (query layout, d-major)."""
    T = qpc // P
    nbr = neighbors.astype(np.int64).reshape(T, P, NN)
    p = np.arange(P)
    g = np.arange(NN)
    # edge-slot permutation: ie[t, p, g] = nbr[t, 4g + p//32, p%32]
    ie = nbr[:, (4 * g[None, :] + p[:, None] // 32), (p[:, None] % 32)]
    nfg = f16[ie]                          # [T, P, NN, F_IN] fp16
    # rel[t, p, d, n] = s[nbr[t,p,n], d] - q[t,p,d]
    qp = query_points.reshape(T, P, DIM)
    relg = (spT[:, nbr].transpose(1, 2, 0, 3) - qp[:, :, :, None]).astype(
        np.float16
    )  # [T, P, 3, NN]
    # edge-slot relative coords: rele[t, p, d, g] = s[ie[t,p,g]] - q[4g+p//32]
    qidx = 4 * g[None, :] + p[:, None] // 32  # [P, NN]
    rele = (
        spT[:, ie].transpose(1, 2, 0, 3) - qp[:, qidx].transpose(0, 1, 3, 2)
    ).astype(np.float16)  # [T, P, 3, NN]
    return nfg, relg, rele


def build_in_maps(query_points, support_points, neighbors, features, K_points,
                  weight, deformable_weight, bias):
    qpc = N_Q // N_CORES
    f16, spT, kprep, dwsb, wsb, brep = _prep_shared(
        support_points, features, K_points, weight, deformable_weight, bias)
    in_maps = []
    for c in range(N_CORES):
        sl = slice(c * qpc, (c + 1) * qpc)
        nfg, relg, rele = _prep_core(query_points[sl], np.asarray(neighbors)[sl],
                                     qpc, f16, spT)
        in_maps.append({
            "nfg": nfg, "relg": relg, "rele": rele,
            "kprep": kprep, "dwsb": dwsb, "wsb": wsb, "brep": brep,
        })
    return qpc, in_maps


def kernel(query_points, support_points, neighbors, features, K_points,
           weight, deformable_weight, bias):
    from concourse.bass_utils import run_bass_kernel_spmd

    query_points = np.asarray(query_points, dtype=np.float32)
    support_points = np.asarray(support_points, dtype=np.float32)
    neighbors = np.asarray(neighbors)
    features = np.asarray(features, dtype=np.float32)
    K_points = np.asarray(K_points, dtype=np.float32)
    weight = np.asarray(weight, dtype=np.float32)
    deformable_weight = np.asarray(deformable_weight, dtype=np.float32)
    bias = np.asarray(bias, dtype=np.float32)

    qpc, in_maps = build_in_maps(
        query_points, support_points, neighbors, features, K_points,
        weight, deformable_weight, bias)
    nc = build_nc(qpc)
    res = run_bass_kernel_spmd(nc, in_maps, core_ids=list(range(N_CORES)))
    out = np.concatenate([res.results[c]["out"] for c in range(N_CORES)], axis=0)
    return out.astype(np.float32)


# revision 17
# speedup vs baseline: 1.4450x; 1.4450x over previous
"""Deformable KPConv layer on 8 Trainium2 NeuronCores (Bass/Tile).

Strategy (data-parallel over the 16384 query points, 2048/core):
  - features pre-cast to fp16 host-side and pregathered per-edge into an
    "edge-slot" layout [(4 queries x 32 neighbors) partitions, group, 128 feat].
  - relative neighbor coords (s - q) prepacked fp16 d-major host-side in
    query-partition layout; squared distances to the (possibly deformed)
    kernel points are computed difference-first (|rel - C|^2 with
    C = kp + offset) on DVE/GpSimd in fp16 -- no cancellation, sq >= 0 by
    construction. Influence w' = min(d,2) - 2 (sign+1/2 folded into the
    conv weights host-side).
  - neighbor contraction on TensorE as block-diagonal matmuls: w' is
    scattered into a zero-initialized block-diagonal SBUF tile via a DRAM
    bounce (partition remap), then psum[f,(q,k)] = nf^T @ wblk.
  - PSUM drains are flat contiguous copies on the Scalar engine into
    wf[f, q, k]; the (k,f)->42 offset projection and (k,f)->256 output
    projection are PSUM-accumulated matmuls with strided wf[:, :, k] views
    as stationary operands, producing query-partition outputs directly.
  - emission is software-pipelined two deep: stage 1 of tile t-1 is
    interleaved after stage 0 of tile t, so in-order engine queues always
    have ready work behind a cross-stage dependency wait.
"""

import os
import sys

sys.path.insert(0, "/opt/trn_rl_repo")

import numpy as np

import concourse.bass as bass
import concourse.tile as tile
from concourse import bacc, mybir

N_Q = 16384
N_S = 16384
NN = 32
F_IN = 128
F_OUT = 256
K = 15
DIM = 3
OFF_DIM = DIM * (K - 1)  # 42
N_CORES = 8
P = 128

F16 = mybir.dt.float16
F32 = mybir.dt.float32


def build_nc(qpc: int):
    T = qpc // P  # query tiles per core
    NG = P // 4  # 32 groups of 4 queries per tile

    nc = bacc.Bacc("TRN2", target_bir_lowering=False)

    nfg_d = nc.dram_tensor("nfg", [T, P, NN, F_IN], F16, kind="ExternalInput")
    relg_d = nc.dram_tensor("relg", [T, P, DIM, NN], F16, kind="ExternalInput")
    rele_d = nc.dram_tensor("rele", [T, P, DIM, NN], F16, kind="ExternalInput")
    kprep_d = nc.dram_tensor("kprep", [P, K * DIM], F32, kind="ExternalInput")
    dwsb_d = nc.dram_tensor("dwsb", [P, K * OFF_DIM], F16, kind="ExternalInput")
    wsb_d = nc.dram_tensor("wsb", [P, K * F_OUT], F16, kind="ExternalInput")
    brep_d = nc.dram_tensor("brep", [P, OFF_DIM], F32, kind="ExternalInput")
    out_d = nc.dram_tensor("out", [qpc, F_OUT], F32, kind="ExternalOutput")

    with tile.TileContext(nc) as tc:
        with (
            tc.tile_pool(name="const", bufs=1) as cpool,
            tc.tile_pool(name="nf", bufs=5) as nfpool,
            tc.tile_pool(name="rl", bufs=4) as rlpool,
            tc.tile_pool(name="sq", bufs=3) as sqpool,
            tc.tile_pool(name="wd", bufs=3) as wdpool,
            tc.tile_pool(name="wf", bufs=3) as wfpool,
            tc.tile_pool(name="cc", bufs=3) as ccpool,
            tc.tile_pool(name="outp", bufs=2) as opool,
            tc.tile_pool(name="dram", bufs=6, space="DRAM") as drpool,
            tc.tile_pool(name="ps", bufs=3, space="PSUM") as pspool,
            tc.tile_pool(name="ps2", bufs=2, space="PSUM") as ps2pool,
        ):
            # --- constants, loaded once ---
            kprep = cpool.tile([P, K, DIM], F32, tag="kprep")
            nc.sync.dma_start(out=kprep[:], in_=kprep_d[:].rearrange("p (k d) -> p k d", d=DIM))
            dwsb = cpool.tile([P, K * OFF_DIM], F16, tag="dwsb")
            nc.sync.dma_start(out=dwsb[:], in_=dwsb_d[:])
            wsb = cpool.tile([P, K * F_OUT], F16, tag="wsb")
            nc.sync.dma_start(out=wsb[:], in_=wsb_d[:])
            brep = cpool.tile([P, OFF_DIM], F32, tag="brep")
            nc.sync.dma_start(out=brep[:], in_=brep_d[:])
            eps_c = cpool.tile([P, 1], F32, tag="eps")
            nc.vector.memset(eps_c[:], 1e-6)
            # fp16 kernel points for the stage-0 (rigid) distances
            kh0 = cpool.tile([P, K, DIM], F16, tag="kh0")
            nc.vector.tensor_copy(out=kh0[:], in_=kprep[:])

            # persistent block-diagonal tiles (zeros off-diagonal; only the
            # diagonal blocks are ever overwritten by the scatter DMAs)
            wblks = []
            for i in range(6):
                wb = nc.alloc_sbuf_tensor(f"wblk{i}", [P, NG, 4 * K], F16)
                nc.gpsimd.memset(wb.ap(), 0.0)
                wblks.append(wb)

            # per-tile state carried between pipeline phases
            state = {}

            def emit_load(t):
                nf = nfpool.tile([P, NN, F_IN], F16, tag="nf")
                nc.sync.dma_start(out=nf[:], in_=nfg_d[t])
                rl = rlpool.tile([P, DIM, NN], F16, tag="rl")
                nc.sync.dma_start(out=rl[:], in_=relg_d[t])
                state[t] = {"nf": nf, "rl": rl}

            def emit_sqrtdist(t, rl, ch):
                """fp16 |rel - C|^2 -> d = sqrt(sq) on ScalarE.
                rel rows broadcast over k, C columns broadcast over n/g."""

                def bc_rel(d):
                    return rl[:, d, :].unsqueeze(2).broadcast_to([P, NN, K])

                def bc_c(d):
                    return ch[:, :, d].unsqueeze(1).broadcast_to([P, NN, K])

                dx = sqpool.tile([P, NN, K], F16, tag="dx")
                nc.gpsimd.tensor_tensor(
                    out=dx[:], in0=bc_rel(0), in1=bc_c(0), op=mybir.AluOpType.subtract
                )
                dy = sqpool.tile([P, NN, K], F16, tag="dy")
                nc.gpsimd.tensor_tensor(
                    out=dy[:], in0=bc_rel(1), in1=bc_c(1), op=mybir.AluOpType.subtract
                )
                dz = sqpool.tile([P, NN, K], F16, tag="dz")
                nc.vector.tensor_tensor(
                    out=dz[:], in0=bc_rel(2), in1=bc_c(2), op=mybir.AluOpType.subtract
                )
                mx = sqpool.tile([P, NN, K], F16, tag="mx")
                nc.vector.tensor_tensor(
                    out=mx[:], in0=dx[:], in1=dx[:], op=mybir.AluOpType.mult
                )
                my = sqpool.tile([P, NN, K], F16, tag="my")
                nc.vector.tensor_tensor(
                    out=my[:], in0=dy[:], in1=dy[:], op=mybir.AluOpType.mult
                )
                mz = sqpool.tile([P, NN, K], F16, tag="mz")
                nc.vector.tensor_tensor(
                    out=mz[:], in0=dz[:], in1=dz[:], op=mybir.AluOpType.mult
                )
                a1 = sqpool.tile([P, NN, K], F16, tag="a1")
                nc.gpsimd.tensor_tensor(
                    out=a1[:], in0=mx[:], in1=my[:], op=mybir.AluOpType.add
                )
                sqt = sqpool.tile([P, NN, K], F16, tag="sqt")
                nc.vector.tensor_tensor(
                    out=sqt[:], in0=a1[:], in1=mz[:], op=mybir.AluOpType.add
                )
                dts = wdpool.tile([P, NN, K], F16, tag="dts")
                nc.scalar.activation(
                    out=dts[:], in_=sqt[:],
                    func=mybir.ActivationFunctionType.Sqrt, bias=eps_c[:],
                )
                return dts

            def emit_dists0(t):
                dts = emit_sqrtdist(t, state[t]["rl"], kh0)
                wdense = wdpool.tile([P, NN * K], F16, tag="wdense")
                nc.vector.tensor_scalar(
                    out=wdense[:],
                    in0=dts[:].rearrange("p n k -> p (n k)"),
                    scalar1=2.0,
                    scalar2=2.0,
                    op0=mybir.AluOpType.min,
                    op1=mybir.AluOpType.subtract,
                )
                wblk = wblks[t % 3].ap()
                bounce = drpool.tile([P, NN * K], F16, tag="bounce")
                nc.scalar.dma_start(out=bounce[:], in_=wdense[:])
                wsrc = bounce[:].rearrange("(g qq) (n k) -> qq n g k", qq=4, k=K)
                for qq in range(4):
                    nc.sync.dma_start(
                        out=wblk[32 * qq : 32 * (qq + 1), :, K * qq : K * (qq + 1)],
                        in_=wsrc[qq],
                    )
                return wblk

            def emit_dists1(t, ch):
                """Stage 1: per-query C -> q-layout compute + DRAM-bounce
                scatter into the block-diagonal tile."""
                dts = emit_sqrtdist(t, state[t]["rl"], ch)
                wdense = wdpool.tile([P, NN * K], F16, tag="wdense")
                nc.vector.tensor_scalar(
                    out=wdense[:],
                    in0=dts[:].rearrange("p n k -> p (n k)"),
                    scalar1=2.0,
                    scalar2=2.0,
                    op0=mybir.AluOpType.min,
                    op1=mybir.AluOpType.subtract,
                )
                wblk = wblks[3 + (t % 3)].ap()
                bounce = drpool.tile([P, NN * K], F16, tag="bounce")
                nc.scalar.dma_start(out=bounce[:], in_=wdense[:])
                wsrc = bounce[:].rearrange("(g qq) (n k) -> qq n g k", qq=4, k=K)
                for qq in range(4):
                    nc.sync.dma_start(
                        out=wblk[32 * qq : 32 * (qq + 1), :, K * qq : K * (qq + 1)],
                        in_=wsrc[qq],
                    )
                return wblk

            def emit_contract(t, stage, wblk):
                """psum[f, (q-in-block, k)] = nf^T . wblk, drained to
                wf[f, q, k] (flat copies on the Scalar engine)."""
                nf = state[t]["nf"]
                wf_sb = wfpool.tile([P, P, K], F16, tag=f"wf{stage}")
                for b in range(4):
                    psb = pspool.tile([P, 8 * 4 * K], F32, tag="psb")
                    for g8 in range(8):
                        g = b * 8 + g8
                        nc.tensor.matmul(
                            out=psb[:, g8 * 60 : (g8 + 1) * 60],
                            lhsT=nf[:, g, :],
                            rhs=wblk[:, g, :],
                            start=True,
                            stop=True,
                        )
                    nc.scalar.activation(
                        out=wf_sb[:, 32 * b : 32 * (b + 1), :],
                        in_=psb[:].rearrange("p (q k) -> p q k", k=K),
                        func=mybir.ActivationFunctionType.Copy,
                    )
                return wf_sb

            def phase_A(t):
                emit_load(t)
                state[t]["wblk0"] = emit_dists0(t)

            def phase_B(t):
                wf0 = emit_contract(t, 0, state[t].pop("wblk0"))
                # offset projection: feat0[q, o] = sum_k wf0_k^T . dw_k
                psA = ps2pool.tile([P, OFF_DIM], F32, tag="psA")
                for k in range(K):
                    nc.tensor.matmul(
                        out=psA[:],
                        lhsT=wf0[:, :, k],
                        rhs=dwsb[:, k * OFF_DIM : (k + 1) * OFF_DIM],
                        start=(k == 0),
                        stop=(k == K - 1),
                    )
                state[t]["psA"] = psA

            def phase_C(t):
                # C1 = kp + offsets (k=0 offset is zero); offsets stay f32
                psA = state[t].pop("psA")
                off_sb = ccpool.tile([P, OFF_DIM], F32, tag="off")
                nc.vector.tensor_tensor(
                    out=off_sb[:], in0=psA[:], in1=brep[:], op=mybir.AluOpType.add
                )
                c1 = ccpool.tile([P, K, DIM], F32, tag="c1")
                nc.vector.tensor_copy(out=c1[:, 0, :], in_=kprep[:, 0, :])
                nc.vector.tensor_tensor(
                    out=c1[:, 1:K, :],
                    in0=kprep[:, 1:K, :],
                    in1=off_sb[:].rearrange("p (k d) -> p k d", d=DIM),
                    op=mybir.AluOpType.add,
                )
                ch1 = ccpool.tile([P, K, DIM], F16, tag="ch1")
                nc.vector.tensor_copy(out=ch1[:], in_=c1[:])
                state[t]["wblk1"] = emit_dists1(t, ch1)

            def phase_D(t):
                wf1 = emit_contract(t, 1, state[t].pop("wblk1"))
                # output projection: out[q, o] = sum_k wf1_k^T . W_k
                psO = ps2pool.tile([P, F_OUT], F32, tag="psO")
                for k in range(K):
                    nc.tensor.matmul(
                        out=psO[:],
                        lhsT=wf1[:, :, k],
                        rhs=wsb[:, k * F_OUT : (k + 1) * F_OUT],
                        start=(k == 0),
                        stop=(k == K - 1),
                    )
                out_sb = opool.tile([P, F_OUT], F32, tag="outsb")
                nc.scalar.activation(
                    out=out_sb[:], in_=psO[:],
                    func=mybir.ActivationFunctionType.Copy,
                )
                nc.scalar.dma_start(
                    out=out_d[t * P : (t + 1) * P, :], in_=out_sb[:]
                )
                del state[t]

            # four-deep software pipeline over tiles; oldest phase emitted
            # first so fresh semaphore waits sit at queue tails
            for step in range(T + 3):
                if 3 <= step < T + 3:
                    phase_D(step - 3)
                if 2 <= step < T + 2:
                    phase_C(step - 2)
                if 1 <= step < T + 1:
                    phase_B(step - 1)
                if step < T:
                    phase_A(step)

    nc.compile()
    return nc


def _prep_shared(support_points, features, K_points, weight, deformable_weight, bias):
    f16 = features.astype(np.float16)
    spT = support_points.T.astype(np.float32)  # [3, N_S]
    kprep = np.broadcast_to(
        K_points.reshape(1, K * DIM), (P, K * DIM)
    ).astype(np.float32).copy()
    dwsb = (
        deformable_weight.transpose(1, 0, 2).reshape(F_IN, K * OFF_DIM) * -0.5
    ).astype(np.float16)
    wsb = (
        weight.transpose(1, 0, 2).reshape(F_IN, K * F_OUT) * -0.5
    ).astype(np.float16)
    brep = np.broadcast_to(bias.reshape(1, OFF_DIM), (P, OFF_DIM)).astype(
        np.float32
    ).copy()
    return f16, spT, kprep, dwsb, wsb, brep


def _prep_core(query_points, neighbors, qpc, f16, spT):
    """Shard-local tensors: pregathered neighbor features (edge-slot layout)
    and relative neighbor coords (query layout, d-major)."""
    T = qpc // P
    nbr = neighbors.astype(np.int64).reshape(T, P, NN)
    p = np.arange(P)
    g = np.arange(NN)
    # edge-slot permutation: ie[t, p, g] = nbr[t, 4g + p//32, p%32]
    ie = nbr[:, (4 * g[None, :] + p[:, None] // 32), (p[:, None] % 32)]
    nfg = f16[ie]                          # [T, P, NN, F_IN] fp16
    # rel[t, p, d, n] = s[nbr[t,p,n], d] - q[t,p,d]
    qp = query_points.reshape(T, P, DIM)
    relg = (spT[:, nbr].transpose(1, 2, 0, 3) - qp[:, :, :, None]).astype(
        np.float16
    )  # [T, P, 3, NN]
    # edge-slot relative coords: rele[t, p, d, g] = s[ie[t,p,g]] - q[4g+p//32]
    qidx = 4 * g[None, :] + p[:, None] // 32  # [P, NN]
    rele = (
        spT[:, ie].transpose(1, 2, 0, 3) - qp[:, qidx].transpose(0, 1, 3, 2)
    ).astype(np.float16)  # [T, P, 3, NN]
    return nfg, relg, rele


def build_in_maps(query_points, support_points, neighbors, features, K_points,
                  weight, deformable_weight, bias):
    qpc = N_Q // N_CORES
    f16, spT, kprep, dwsb, wsb, brep = _prep_shared(
        support_points, features, K_points, weight, deformable_weight, bias)
    in_maps = []
    for c in range(N_CORES):
        sl = slice(c * qpc, (c + 1) * qpc)
        nfg, relg, rele = _prep_core(query_points[sl], np.asarray(neighbors)[sl],
                                     qpc, f16, spT)
        in_maps.append({
            "nfg": nfg, "relg": relg, "rele": rele,
            "kprep": kprep, "dwsb": dwsb, "wsb": wsb, "brep": brep,
        })
    return qpc, in_maps


def kernel(query_points, support_points, neighbors, features, K_points,
           weight, deformable_weight, bias):
    from concourse.bass_utils import run_bass_kernel_spmd

    query_points = np.asarray(query_points, dtype=np.float32)
    support_points = np.asarray(support_points, dtype=np.float32)
    neighbors = np.asarray(neighbors)
    features = np.asarray(features, dtype=np.float32)
    K_points = np.asarray(K_points, dtype=np.float32)
    weight = np.asarray(weight, dtype=np.float32)
    deformable_weight = np.asarray(deformable_weight, dtype=np.float32)
    bias = np.asarray(bias, dtype=np.float32)

    qpc, in_maps = build_in_maps(
        query_points, support_points, neighbors, features, K_points,
        weight, deformable_weight, bias)
    nc = build_nc(qpc)
    res = run_bass_kernel_spmd(nc, in_maps, core_ids=list(range(N_CORES)))
    out = np.concatenate([res.results[c]["out"] for c in range(N_CORES)], axis=0)
    return out.astype(np.float32)
